# revision 33
# baseline (speedup 1.0000x reference)
"""GQA attention (B=2,T=2048,D=2048,H=16,KV=4,HD=128, causal+RoPE) on 8 trn2 cores.

Sharding: 4-way head tensor-parallel x 2-way batch data-parallel.
Core c: batch b=c//4, TP shard s=c%4 -> q heads [4s..4s+3], kv head s.

Transposed-scores flash attention: S^T[k, q] = kT^T qT per 128-key
block x 512-query chunk; exp on the scalar engine writes P^T straight
to SBUF in fp8, so the PV matmul consumes it as the moving operand
(mixed bf16xfp8) with no PE transposes and no PSUM->SBUF copies.
Diagonal blocks compute only their valid q-range (narrowed S^T / exp /
mask / PV / l).  Causal mask = affine_select on GpSimd zeroing the
invalid triangle of P^T in SBUF; diagonal blocks run FIRST in each
unit so the mask chain overlaps the unit's remaining pipeline.
Softmax denominator l: fp8 DoubleRow all-ones matmul per off-diagonal
PT pair (0.5 cycles/row), narrowed fp8 matmuls for diagonal blocks.
1/l on DVE; partition-broadcast on GpSimd; one DVE multiply normalizes
O^T, which feeds the Wo matmul directly as lhsT.  The 1/sqrt(HD) scale
is folded into Wq host-side so q and k share one pair of RoPE tables.
Projection and output-projection work is interleaved unit-by-unit into
the attention stream so the PE fills the slack while the scalar engine
streams exps.  Input DMAs are spread across the sync/gpsimd/scalar
engine queues to parallelize HBM streams.
"""

import math
import os
import numpy as np

try:
    import concourse.bass as bass
except ImportError:  # pragma: no cover
    import sys

    sys.path.insert(0, "/opt/trn_rl_repo")
    import concourse.bass as bass

import concourse.mybir as mybir
import concourse.bacc as bacc
from concourse import bass_utils
from concourse.tile import TileContext
from contextlib import ExitStack
from ml_dtypes import bfloat16

B, T, D = 2, 2048, 2048
H, KV, HD = 16, 4, 128
TP = 4  # head-TP ways
NH = H // TP  # q heads per core = 4
NKB = D // 128  # 16 contraction blocks
NQC = T // 512  # 4 query chunks
NTB = T // 128  # 16 token blocks
SCALE = 1.0 / math.sqrt(HD)
F32 = mybir.dt.float32
BF16 = mybir.dt.bfloat16
FP8 = mybir.dt.float8e4
EXP = mybir.ActivationFunctionType.Exp
DR = mybir.MatmulPerfMode.DoubleRow

_program = None
_last_results = None
last_exec_time_ns = None


def _build_program():
    global _program
    if _program is not None:
        return _program

    nc = bacc.Bacc(
        "TRN2",
        target_bir_lowering=False,
        debug=False,
        enable_asserts=False,
        num_devices=8,
    )
    xT_d = nc.dram_tensor("xT", [D, T], BF16, kind="ExternalInput").ap()
    # weights pre-laid-out host side to match SBUF tiles
    wq_d = nc.dram_tensor("Wq", [128, NH * NKB * HD], BF16, kind="ExternalInput").ap()
    wk_d = nc.dram_tensor("Wk", [128, NKB * HD], BF16, kind="ExternalInput").ap()
    wv_d = nc.dram_tensor("Wv", [128, NKB * HD], BF16, kind="ExternalInput").ap()
    wo_d = nc.dram_tensor("Wo", [128, NH * D], BF16, kind="ExternalInput").ap()
    ck_d = nc.dram_tensor("cosk", [128, T], BF16, kind="ExternalInput").ap()
    sk_d = nc.dram_tensor("sink", [128, T], BF16, kind="ExternalInput").ap()
    y_d = nc.dram_tensor("y", [T, D], BF16, kind="ExternalOutput").ap()

    with TileContext(nc) as tc, ExitStack() as ctx:
        big = ctx.enter_context(tc.tile_pool(name="big", bufs=1))
        ps = ctx.enter_context(tc.tile_pool(name="ps", bufs=2, space="PSUM"))
        stp = ctx.enter_context(tc.tile_pool(name="stp", bufs=3, space="PSUM"))
        otp = ctx.enter_context(tc.tile_pool(name="otp", bufs=2, space="PSUM"))
        lp = ctx.enter_context(tc.tile_pool(name="lp", bufs=1, space="PSUM"))
        ropep = ctx.enter_context(tc.tile_pool(name="ropep", bufs=2))
        ptp = ctx.enter_context(tc.tile_pool(name="ptp", bufs=3))
        rlp = ctx.enter_context(tc.tile_pool(name="rlp", bufs=2))
        yp = ctx.enter_context(tc.tile_pool(name="yp", bufs=3))

        xT = big.tile([128, NKB, T], BF16, tag="xT")
        # one tile per head so each head's first matmul depends only on its
        # own DMA (a shared tile coarsens the dependency to all four DMAs)
        wqh = [
            big.tile([128, NKB, HD], BF16, tag=f"wq{h}", name=f"wq{h}")
            for h in range(NH)
        ]
        wk = big.tile([128, NKB, HD], BF16, tag="wk")
        wv = big.tile([128, NKB, HD], BF16, tag="wv")
        wo = big.tile([128, NH, D], BF16, tag="wo")
        ck = big.tile([128, T], BF16, tag="ck")
        sk = big.tile([128, T], BF16, tag="sk")
        ones8 = big.tile([128, 2, 128], FP8, tag="ones8")
        bias1 = big.tile([128, 1], F32, tag="bias1")
        nc.vector.memset(ones8[:], 1.0)
        nc.vector.memset(bias1[:], -1.0)
        qT = big.tile([128, NH, T], BF16, tag="qT")
        kT = big.tile([128, T], BF16, tag="kT")
        V = big.tile([128, NTB, HD], BF16, tag="V")
        OT = big.tile([128, NH, T], BF16, tag="OT")

        # ---- loads; spread across engine DMA queues so streams parallelize.
        # sync: xT (8 MB).  gpsimd: wk, rope tables, wv, wq (head-major).
        # scalar: wo.
        for q in range(4):
            nc.gpsimd.dma_start(
                out=wk[:, 4 * q : 4 * q + 4, :],
                in_=wk_d[:, q * 512 : (q + 1) * 512],
            )
        for j in range(NKB):
            eng = nc.sync if j % 2 == 0 else nc.scalar
            eng.dma_start(
                out=xT[:, j, 0:512], in_=xT_d[j * 128 : (j + 1) * 128, 0:512]
            )
        for d_ap, s_tile in ((ck_d, ck), (sk_d, sk)):
            nc.gpsimd.dma_start(out=s_tile[:, :1024], in_=d_ap[:, :1024])
            nc.gpsimd.dma_start(out=s_tile[:, 1024:], in_=d_ap[:, 1024:])
        for h in range(NH):
            nc.gpsimd.dma_start(
                out=wqh[h][:, :, :], in_=wq_d[:, h * 2048 : (h + 1) * 2048]
            )
        for q in range(4):
            nc.gpsimd.dma_start(
                out=wv[:, 4 * q : 4 * q + 4, :],
                in_=wv_d[:, q * 512 : (q + 1) * 512],
            )
        # remaining xT per-chunk in chunk order so chunk c's projections can
        # start as soon as chunk c lands; wo (needed ~t=45us) slots between
        # c2 and c3 on the scalar queue
        for c in range(1, NQC):
            for j in range(NKB):
                eng = nc.sync if j % 2 == 0 else nc.scalar
                eng.dma_start(
                    out=xT[:, j, c * 512 : (c + 1) * 512],
                    in_=xT_d[j * 128 : (j + 1) * 128, c * 512 : (c + 1) * 512],
                )
            if c == 2:
                for q in range(8):
                    nc.scalar.dma_start(
                        out=wo[:, q // 2, (q % 2) * 1024 : (q % 2) * 1024 + 1024],
                        in_=wo_d[:, q * 1024 : (q + 1) * 1024],
                    )

        def rope_unit(w_lhsT_of_j, dst, c):
            sl = slice(c * 512, (c + 1) * 512)
            pst = ps.tile([128, 512], F32, tag="ps")
            for j in range(NKB):
                nc.tensor.matmul(
                    pst[:],
                    lhsT=w_lhsT_of_j(j),
                    rhs=xT[:, j, sl],
                    start=(j == 0),
                    stop=(j == NKB - 1),
                )
            ri = ropep.tile([128, 512], BF16, tag="ri")
            nc.scalar.copy(ri[:], pst[:])
            t1 = ropep.tile([128, 512], BF16, tag="t1")
            t2 = ropep.tile([128, 512], BF16, tag="t2")
            # sin table halves pre-swapped host-side so each mul's two SBUF
            # inputs share a base partition (BIR verifier requirement)
            nc.vector.tensor_mul(t1[:], ri[:], ck[:, sl])
            nc.vector.tensor_mul(t2[0:64, :], ri[64:128, :], sk[64:128, sl])
            nc.vector.tensor_mul(t2[64:128, :], ri[0:64, :], sk[0:64, sl])
            nc.vector.tensor_add(dst[:, sl], t1[:], t2[:])

        def v_unit(tb):
            pst = ps.tile([128, 512], F32, tag="ps")
            for j in range(NKB):
                nc.tensor.matmul(
                    pst[:, 0:128],
                    lhsT=xT[:, j, tb * 128 : (tb + 1) * 128],
                    rhs=wv[:, j, :],
                    start=(j == 0),
                    stop=(j == NKB - 1),
                )
            nc.vector.tensor_copy(V[:, tb, :], pst[:, 0:128])

        def proj_units(c):
            units = [lambda: rope_unit(lambda j: wk[:, j, :], kT, c)]
            for h in range(NH):
                units.append(
                    lambda h=h: rope_unit(lambda j: wqh[h][:, j, :], qT[:, h, :], c)
                )
            for tb in range(4 * c, 4 * c + 4):
                units.append(lambda tb=tb: v_unit(tb))
            return units

        def attn_unit(qc, h):
            qsl = slice(qc * 512, (qc + 1) * 512)
            nkb = 4 * (qc + 1)
            # diagonal blocks first so their mask chain overlaps the rest
            order = list(range(4 * qc, nkb)) + list(range(0, 4 * qc))
            ot_ps = otp.tile([128, 512], F32, tag="ot")
            l_ps = lp.tile([128, 512], F32, tag="l")
            pt = None
            for idx, kb in enumerate(order):
                diag = kb >= 4 * qc
                jd = kb - 4 * qc if diag else 0
                off = 128 * jd  # first valid q column within the chunk
                w = 512 - off
                if idx % 2 == 0:
                    pt = ptp.tile([128, 2, 512], FP8, tag="pt")
                psl = pt[:, idx % 2, off:512]
                st = stp.tile([128, 512], F32, tag="st")
                nc.tensor.matmul(
                    st[:, 0:w],
                    lhsT=kT[:, kb * 128 : (kb + 1) * 128],
                    rhs=qT[:, h, qc * 512 + off : (qc + 1) * 512],
                    start=True,
                    stop=True,
                )
                # exp(s - 1): bias keeps P^T in fp8's normal range
                # (cancels in the softmax normalization)
                nc.scalar.activation(psl, st[:, 0:w], EXP, bias=bias1[:])
                if diag:
                    # zero where q < k within the diagonal block
                    nc.gpsimd.affine_select(
                        out=psl,
                        in_=psl,
                        pattern=[[1, w]],
                        compare_op=mybir.AluOpType.is_ge,
                        fill=0.0,
                        base=0,
                        channel_multiplier=-1,
                    )
                nc.tensor.matmul(
                    ot_ps[:, off:512],
                    lhsT=V[:, kb, :],
                    rhs=psl,
                    start=(idx == 0),
                    stop=(idx == nkb - 1),
                )
                if diag:
                    # narrowed fp8 denominator matmul per diagonal block
                    nc.tensor.matmul(
                        l_ps[:, off:512],
                        lhsT=ones8[:, 0, :],
                        rhs=psl,
                        start=(idx == 0),
                        stop=(idx == nkb - 1),
                    )
                elif idx % 2 == 1:
                    # off-diagonal pairs: fp8 DoubleRow all-ones matmul
                    nc.tensor.matmul(
                        l_ps[:],
                        lhsT=ones8[:],
                        rhs=pt[:, :, :],
                        start=False,
                        stop=(idx == nkb - 1),
                        perf_mode=DR,
                    )
            # fast 1-op approx reciprocal (~18 bits, result feeds bf16 anyway);
            # the exact DVE reciprocal is ~3.3us and clogs the vector queue
            rl = rlp.tile([128, 512], F32, tag="rl")
            nc.vector.reciprocal_approx_fast(rl[0:1, :], l_ps[0:1, :])
            bc = rlp.tile([128, 512], F32, tag="bc")
            nc.gpsimd.partition_broadcast(bc[:], rl[0:1, :])
            nc.vector.tensor_mul(OT[:, h, qsl], ot_ps[:], bc[:])

        def wo_unit(tb, dc, last=False):
            yps = ps.tile([128, 512], F32, tag="ps")
            for h in range(NH):
                nc.tensor.matmul(
                    yps[:],
                    lhsT=OT[:, h, tb * 128 : (tb + 1) * 128],
                    rhs=wo[:, h, dc * 512 : (dc + 1) * 512],
                    start=(h == 0),
                    stop=(h == NH - 1),
                )
            ysb = yp.tile([128, 512], BF16, tag="y")
            if dc % 2 == 0:
                nc.vector.tensor_copy(ysb[:], yps[:])
            else:
                nc.scalar.copy(ysb[:], yps[:])
            # stores stay off the gpsimd queue (it carries the mask /
            # broadcast chain); the final chunk splits each store across
            # two queues to shorten the drain tail
            if last:
                for sp, eng in ((0, nc.sync), (1, nc.gpsimd)):
                    eng.dma_start(
                        out=y_d[
                            tb * 128 : (tb + 1) * 128,
                            dc * 512 + sp * 256 : dc * 512 + sp * 256 + 256,
                        ],
                        in_=ysb[:, sp * 256 : sp * 256 + 256],
                    )
            else:
                nc.sync.dma_start(
                    out=y_d[tb * 128 : (tb + 1) * 128, dc * 512 : (dc + 1) * 512],
                    in_=ysb[:],
                )

        # interleave: after each attention unit, emit a couple of next-chunk
        # projection units and one token-block of previous-chunk Wo units
        PROJ_SPLIT = [2, 2, 2, 3]  # 9 proj units spread over 4 attn units
        for u in proj_units(0):
            u()
        for qc in range(NQC):
            fillers = proj_units(qc + 1) if qc + 1 < NQC else []
            fi = 0
            for h in range(NH):
                attn_unit(qc, h)
                for _ in range(PROJ_SPLIT[h]):
                    if fi < len(fillers):
                        fillers[fi]()
                        fi += 1
                if qc > 0:
                    tb = 4 * (qc - 1) + h
                    for dc in range(4):
                        wo_unit(tb, dc)
            while fi < len(fillers):
                fillers[fi]()
                fi += 1
        for tb in range(12, 16):
            for dc in range(4):
                wo_unit(tb, dc, last=True)

    nc.compile()
    _program = nc
    return nc


def _host_prep(x, Wq, Wk, Wv, Wo):
    x = np.asarray(x, dtype=np.float32)
    Wq = np.asarray(Wq, dtype=np.float32) * SCALE  # fold softmax scale into Wq
    Wk = np.asarray(Wk, dtype=np.float32)
    Wv = np.asarray(Wv, dtype=np.float32)
    Wo = np.asarray(Wo, dtype=np.float32)

    # RoPE even/odd gather folded into weight column permutation (per head)
    perm128 = np.r_[np.arange(0, 128, 2), np.arange(1, 128, 2)]
    permq = np.concatenate([hb * 128 + perm128 for hb in range(H)])
    permk = np.concatenate([hb * 128 + perm128 for hb in range(KV)])
    Wq_p = Wq[:, permq]
    Wk_p = Wk[:, permk]

    pos = np.arange(T, dtype=np.float64)
    inv_freq = 1.0 / (10000.0 ** (np.arange(0, HD, 2, dtype=np.float64) / HD))
    ang = np.einsum("t,f->tf", pos, inv_freq)  # [T, 64]
    cos = np.cos(ang).T  # [64, T]
    sin = np.sin(ang).T
    cosk = np.concatenate([cos, cos], axis=0).astype(bfloat16)  # [128, T]
    # halves pre-swapped: rows 0:64 hold +sin (odd-half output),
    # rows 64:128 hold -sin (even-half output)
    sink = np.concatenate([sin, -sin], axis=0).astype(bfloat16)

    def sb_layout(w):
        # [D, C] -> [128, NKB*C] matching SBUF tile [128, NKB, C]
        dcols = w.shape[1]
        return np.ascontiguousarray(
            w.reshape(NKB, 128, dcols).transpose(1, 0, 2).reshape(128, NKB * dcols)
        )

    def wq_layout(w):
        # [D, 512] -> [128, NH*NKB*128] head-major matching [128, NH, NKB, 128]
        a = w.reshape(NKB, 128, NH, HD).transpose(1, 2, 0, 3)
        return np.ascontiguousarray(a.reshape(128, NH * NKB * HD))

    def wo_layout(w):
        # [512, D] -> [128, NH*D] matching SBUF tile [128, NH, D]
        return np.ascontiguousarray(
            w.reshape(NH, 128, D).transpose(1, 0, 2).reshape(128, NH * D)
        )

    in_maps = []
    for c in range(8):
        b, s = c // 4, c % 4
        in_maps.append(
            {
                "xT": np.ascontiguousarray(x[b].T).astype(bfloat16),
                "Wq": wq_layout(Wq_p[:, s * 512 : (s + 1) * 512]).astype(bfloat16),
                "Wk": sb_layout(Wk_p[:, s * 128 : (s + 1) * 128]).astype(bfloat16),
                "Wv": sb_layout(Wv[:, s * 128 : (s + 1) * 128]).astype(bfloat16),
                "Wo": wo_layout(Wo[s * 512 : (s + 1) * 512, :]).astype(bfloat16),
                "cosk": cosk,
                "sink": sink,
            }
        )
    return in_maps


def _ensure_ntff_hook():
    """The agent image's antenv lacks axon_hooks, so boot() skips installing
    the NTFF profile hook. Recreate the module and install the hook."""
    import sys
    import types

    try:
        from antenv.axon_hooks import get_axon_ntff_profile_hook  # noqa: F401

        return True
    except ImportError:
        pass
    try:
        import antenv
        from trn_agent_boot.trn_boot import _ntff_profile_via_ctypes

        hook = _ntff_profile_via_ctypes("/opt/axon/libaxon_pjrt.so")
        if hook is None:
            return False
        mod = types.ModuleType("antenv.axon_hooks")
        mod._hook = hook
        mod.set_axon_ntff_profile_hook = lambda h: setattr(mod, "_hook", h)
        mod.get_axon_ntff_profile_hook = lambda: mod._hook
        sys.modules["antenv.axon_hooks"] = mod
        antenv.axon_hooks = mod
        bass_utils.upload_artifacts = lambda d: d
        return True
    except Exception:
        return False


def kernel(x, Wq, Wk, Wv, Wo):
    global _last_results, last_exec_time_ns
    nc = _build_program()
    in_maps = _host_prep(x, Wq, Wk, Wv, Wo)
    trace = bool(int(os.environ.get("KERNEL_TRACE", "0")))
    tmpdir = None
    if trace:
        trace = _ensure_ntff_hook()
        if trace:
            tmpdir = os.environ.get("KERNEL_TRACE_DIR") or None
    res = bass_utils.run_bass_kernel_spmd(
        nc, in_maps, core_ids=list(range(8)), trace=trace, tmpdir=tmpdir
    )
    _last_results = res
    last_exec_time_ns = res.exec_time_ns
    out = np.empty((B, T, D), dtype=np.float32)
    for b in range(B):
        out[b] = sum(
            res.results[4 * b + s]["y"].astype(np.float32) for s in range(TP)
        )
    return out


# revision 36
# speedup vs baseline: 1.0814x; 1.0814x over previous
"""GQA attention (B=2,T=2048,D=2048,H=16,KV=4,HD=128, causal+RoPE) on 8 trn2 cores.

Sharding: 4-way head tensor-parallel x 2-way batch data-parallel.
Core c: batch b=c//4, TP shard s=c%4 -> q heads [4s..4s+3], kv head s.

Transposed-scores flash attention: S^T[k, q] = kT^T qT per 128-key
block x 512-query chunk; exp on the scalar engine writes P^T straight
to SBUF in fp8, so the PV matmul consumes it as the moving operand
(mixed bf16xfp8) with no PE transposes and no PSUM->SBUF copies.
Diagonal blocks compute only their valid q-range (narrowed S^T / exp /
mask / PV / l).  Causal mask = affine_select on GpSimd zeroing the
invalid triangle of P^T in SBUF; diagonal blocks run FIRST in each
unit so the mask chain overlaps the unit's remaining pipeline.
Softmax denominator l: fp8 DoubleRow all-ones matmul per off-diagonal
PT pair (0.5 cycles/row), narrowed fp8 matmuls for diagonal blocks.
1/l on DVE; partition-broadcast on GpSimd; one DVE multiply normalizes
O^T, which feeds the Wo matmul directly as lhsT.  The 1/sqrt(HD) scale
is folded into Wq host-side so q and k share one pair of RoPE tables.
Projection and output-projection work is interleaved unit-by-unit into
the attention stream so the PE fills the slack while the scalar engine
streams exps.  Input DMAs are spread across the sync/gpsimd/scalar
engine queues to parallelize HBM streams.
"""

import math
import os
import numpy as np

try:
    import concourse.bass as bass
except ImportError:  # pragma: no cover
    import sys

    sys.path.insert(0, "/opt/trn_rl_repo")
    import concourse.bass as bass

import concourse.mybir as mybir
import concourse.bacc as bacc
from concourse import bass_utils
from concourse.tile import TileContext
from contextlib import ExitStack
from ml_dtypes import bfloat16

B, T, D = 2, 2048, 2048
H, KV, HD = 16, 4, 128
TP = 4  # head-TP ways
NH = H // TP  # q heads per core = 4
NKB = D // 128  # 16 contraction blocks
NQC = T // 512  # 4 query chunks
NTB = T // 128  # 16 token blocks
SCALE = 1.0 / math.sqrt(HD)
F32 = mybir.dt.float32
BF16 = mybir.dt.bfloat16
FP8 = mybir.dt.float8e4
EXP = mybir.ActivationFunctionType.Exp
DR = mybir.MatmulPerfMode.DoubleRow

_program = None
_last_results = None
last_exec_time_ns = None


def _build_program():
    global _program
    if _program is not None:
        return _program

    nc = bacc.Bacc(
        "TRN2",
        target_bir_lowering=False,
        debug=False,
        enable_asserts=False,
        num_devices=8,
    )
    xT_d = nc.dram_tensor("xT", [D, T], BF16, kind="ExternalInput").ap()
    # weights pre-laid-out host side to match SBUF tiles
    wq_d = nc.dram_tensor("Wq", [128, NH * NKB * HD], BF16, kind="ExternalInput").ap()
    wk_d = nc.dram_tensor("Wk", [128, NKB * HD], BF16, kind="ExternalInput").ap()
    wv_d = nc.dram_tensor("Wv", [128, NKB * HD], BF16, kind="ExternalInput").ap()
    wo_d = nc.dram_tensor("Wo", [128, NH * D], BF16, kind="ExternalInput").ap()
    ck_d = nc.dram_tensor("cosk", [128, T], BF16, kind="ExternalInput").ap()
    sk_d = nc.dram_tensor("sink", [128, T], BF16, kind="ExternalInput").ap()
    y_d = nc.dram_tensor("y", [T, D], BF16, kind="ExternalOutput").ap()

    with TileContext(nc) as tc, ExitStack() as ctx:
        big = ctx.enter_context(tc.tile_pool(name="big", bufs=1))
        ps = ctx.enter_context(tc.tile_pool(name="ps", bufs=2, space="PSUM"))
        stp = ctx.enter_context(tc.tile_pool(name="stp", bufs=3, space="PSUM"))
        otp = ctx.enter_context(tc.tile_pool(name="otp", bufs=2, space="PSUM"))
        lp = ctx.enter_context(tc.tile_pool(name="lp", bufs=1, space="PSUM"))
        ropep = ctx.enter_context(tc.tile_pool(name="ropep", bufs=2))
        ptp = ctx.enter_context(tc.tile_pool(name="ptp", bufs=3))
        rlp = ctx.enter_context(tc.tile_pool(name="rlp", bufs=2))
        yp = ctx.enter_context(tc.tile_pool(name="yp", bufs=3))

        xT = big.tile([128, NKB, T], BF16, tag="xT")
        # one tile per head so each head's first matmul depends only on its
        # own DMA (a shared tile coarsens the dependency to all four DMAs)
        wqh = [
            big.tile([128, NKB, HD], BF16, tag=f"wq{h}", name=f"wq{h}")
            for h in range(NH)
        ]
        wk = big.tile([128, NKB, HD], BF16, tag="wk")
        wv = big.tile([128, NKB, HD], BF16, tag="wv")
        wo = big.tile([128, NH, D], BF16, tag="wo")
        ck = big.tile([128, T], BF16, tag="ck")
        sk = big.tile([128, T], BF16, tag="sk")
        ones8 = big.tile([128, 2, 128], FP8, tag="ones8")
        bias1 = big.tile([128, 1], F32, tag="bias1")
        nc.vector.memset(ones8[:], 1.0)
        nc.vector.memset(bias1[:], -1.0)
        qT = big.tile([128, NH, T], BF16, tag="qT")
        kT = big.tile([128, T], BF16, tag="kT")
        V = big.tile([128, NTB, HD], BF16, tag="V")
        OT = big.tile([128, NH, T], BF16, tag="OT")

        # ---- loads; spread across engine DMA queues so streams parallelize.
        # sync: xT (8 MB).  gpsimd: wk, rope tables, wv, wq (head-major).
        # scalar: wo.
        for q in range(4):
            nc.gpsimd.dma_start(
                out=wk[:, 4 * q : 4 * q + 4, :],
                in_=wk_d[:, q * 512 : (q + 1) * 512],
            )
        # all xT triggers ride the otherwise-idle sync engine: DMA triggers
        # cost ~1us of issuing-engine time, which the scalar engine (exp
        # stream) cannot spare
        for j in range(NKB):
            nc.sync.dma_start(
                out=xT[:, j, 0:512], in_=xT_d[j * 128 : (j + 1) * 128, 0:512]
            )
        for d_ap, s_tile in ((ck_d, ck), (sk_d, sk)):
            nc.gpsimd.dma_start(out=s_tile[:, :1024], in_=d_ap[:, :1024])
            nc.gpsimd.dma_start(out=s_tile[:, 1024:], in_=d_ap[:, 1024:])
        for h in range(NH):
            nc.gpsimd.dma_start(
                out=wqh[h][:, :, :], in_=wq_d[:, h * 2048 : (h + 1) * 2048]
            )
        for q in range(4):
            nc.gpsimd.dma_start(
                out=wv[:, 4 * q : 4 * q + 4, :],
                in_=wv_d[:, q * 512 : (q + 1) * 512],
            )
        for q in range(8):
            nc.scalar.dma_start(
                out=wo[:, q // 2, (q % 2) * 1024 : (q % 2) * 1024 + 1024],
                in_=wo_d[:, q * 1024 : (q + 1) * 1024],
            )
        for j in range(NKB):
            nc.sync.dma_start(
                out=xT[:, j, 512:2048], in_=xT_d[j * 128 : (j + 1) * 128, 512:2048]
            )

        def rope_unit(w_lhsT_of_j, dst, c):
            sl = slice(c * 512, (c + 1) * 512)
            pst = ps.tile([128, 512], F32, tag="ps")
            for j in range(NKB):
                nc.tensor.matmul(
                    pst[:],
                    lhsT=w_lhsT_of_j(j),
                    rhs=xT[:, j, sl],
                    start=(j == 0),
                    stop=(j == NKB - 1),
                )
            ri = ropep.tile([128, 512], BF16, tag="ri")
            nc.scalar.copy(ri[:], pst[:])
            t1 = ropep.tile([128, 512], BF16, tag="t1")
            t2 = ropep.tile([128, 512], BF16, tag="t2")
            # sin table halves pre-swapped host-side so each mul's two SBUF
            # inputs share a base partition (BIR verifier requirement)
            nc.vector.tensor_mul(t1[:], ri[:], ck[:, sl])
            nc.vector.tensor_mul(t2[0:64, :], ri[64:128, :], sk[64:128, sl])
            nc.vector.tensor_mul(t2[64:128, :], ri[0:64, :], sk[0:64, sl])
            nc.vector.tensor_add(dst[:, sl], t1[:], t2[:])

        def v_unit(tb):
            pst = ps.tile([128, 512], F32, tag="ps")
            for j in range(NKB):
                nc.tensor.matmul(
                    pst[:, 0:128],
                    lhsT=xT[:, j, tb * 128 : (tb + 1) * 128],
                    rhs=wv[:, j, :],
                    start=(j == 0),
                    stop=(j == NKB - 1),
                )
            nc.vector.tensor_copy(V[:, tb, :], pst[:, 0:128])

        def proj_units(c):
            units = [lambda: rope_unit(lambda j: wk[:, j, :], kT, c)]
            for h in range(NH):
                units.append(
                    lambda h=h: rope_unit(lambda j: wqh[h][:, j, :], qT[:, h, :], c)
                )
            for tb in range(4 * c, 4 * c + 4):
                units.append(lambda tb=tb: v_unit(tb))
            return units

        def attn_unit(qc, h):
            qsl = slice(qc * 512, (qc + 1) * 512)
            nkb = 4 * (qc + 1)
            # diagonal blocks first so their mask chain overlaps the rest
            order = list(range(4 * qc, nkb)) + list(range(0, 4 * qc))
            ot_ps = otp.tile([128, 512], F32, tag="ot")
            l_ps = lp.tile([128, 512], F32, tag="l")
            pt = None
            for idx, kb in enumerate(order):
                diag = kb >= 4 * qc
                jd = kb - 4 * qc if diag else 0
                off = 128 * jd  # first valid q column within the chunk
                w = 512 - off
                if idx % 2 == 0:
                    pt = ptp.tile([128, 2, 512], FP8, tag="pt")
                psl = pt[:, idx % 2, off:512]
                st = stp.tile([128, 512], F32, tag="st")
                nc.tensor.matmul(
                    st[:, 0:w],
                    lhsT=kT[:, kb * 128 : (kb + 1) * 128],
                    rhs=qT[:, h, qc * 512 + off : (qc + 1) * 512],
                    start=True,
                    stop=True,
                )
                # exp(s - 1): bias keeps P^T in fp8's normal range
                # (cancels in the softmax normalization)
                nc.scalar.activation(psl, st[:, 0:w], EXP, bias=bias1[:])
                if diag:
                    # zero where q < k within the diagonal block
                    nc.gpsimd.affine_select(
                        out=psl,
                        in_=psl,
                        pattern=[[1, w]],
                        compare_op=mybir.AluOpType.is_ge,
                        fill=0.0,
                        base=0,
                        channel_multiplier=-1,
                    )
                nc.tensor.matmul(
                    ot_ps[:, off:512],
                    lhsT=V[:, kb, :],
                    rhs=psl,
                    start=(idx == 0),
                    stop=(idx == nkb - 1),
                )
                if diag:
                    # narrowed fp8 denominator matmul per diagonal block
                    nc.tensor.matmul(
                        l_ps[:, off:512],
                        lhsT=ones8[:, 0, :],
                        rhs=psl,
                        start=(idx == 0),
                        stop=(idx == nkb - 1),
                    )
                elif idx % 2 == 1:
                    # off-diagonal pairs: fp8 DoubleRow all-ones matmul
                    nc.tensor.matmul(
                        l_ps[:],
                        lhsT=ones8[:],
                        rhs=pt[:, :, :],
                        start=False,
                        stop=(idx == nkb - 1),
                        perf_mode=DR,
                    )
            # fast 1-op approx reciprocal (~18 bits, result feeds bf16 anyway);
            # the exact DVE reciprocal is ~3.3us and clogs the vector queue
            rl = rlp.tile([128, 512], F32, tag="rl")
            nc.vector.reciprocal_approx_fast(rl[0:1, :], l_ps[0:1, :])
            bc = rlp.tile([128, 512], F32, tag="bc")
            nc.gpsimd.partition_broadcast(bc[:], rl[0:1, :])
            nc.vector.tensor_mul(OT[:, h, qsl], ot_ps[:], bc[:])

        def wo_unit(tb, dc, last=False):
            yps = ps.tile([128, 512], F32, tag="ps")
            for h in range(NH):
                nc.tensor.matmul(
                    yps[:],
                    lhsT=OT[:, h, tb * 128 : (tb + 1) * 128],
                    rhs=wo[:, h, dc * 512 : (dc + 1) * 512],
                    start=(h == 0),
                    stop=(h == NH - 1),
                )
            ysb = yp.tile([128, 512], BF16, tag="y")
            if dc % 2 == 0:
                nc.vector.tensor_copy(ysb[:], yps[:])
            else:
                nc.scalar.copy(ysb[:], yps[:])
            # stores stay off the gpsimd queue (it carries the mask /
            # broadcast chain); the final chunk splits each store across
            # two queues to shorten the drain tail
            if last:
                for sp, eng in ((0, nc.sync), (1, nc.scalar)):
                    eng.dma_start(
                        out=y_d[
                            tb * 128 : (tb + 1) * 128,
                            dc * 512 + sp * 256 : dc * 512 + sp * 256 + 256,
                        ],
                        in_=ysb[:, sp * 256 : sp * 256 + 256],
                    )
            else:
                nc.sync.dma_start(
                    out=y_d[tb * 128 : (tb + 1) * 128, dc * 512 : (dc + 1) * 512],
                    in_=ysb[:],
                )

        # interleave: after each attention unit, emit a couple of next-chunk
        # projection units and one token-block of previous-chunk Wo units
        PROJ_SPLIT = [2, 2, 2, 3]  # 9 proj units spread over 4 attn units
        for u in proj_units(0):
            u()
        for qc in range(NQC):
            fillers = proj_units(qc + 1) if qc + 1 < NQC else []
            fi = 0
            for h in range(NH):
                attn_unit(qc, h)
                for _ in range(PROJ_SPLIT[h]):
                    if fi < len(fillers):
                        fillers[fi]()
                        fi += 1
                if qc > 0:
                    tb = 4 * (qc - 1) + h
                    for dc in range(4):
                        wo_unit(tb, dc)
            while fi < len(fillers):
                fillers[fi]()
                fi += 1
        for tb in range(12, 16):
            for dc in range(4):
                wo_unit(tb, dc, last=True)

    nc.compile()
    _program = nc
    return nc


def _host_prep(x, Wq, Wk, Wv, Wo):
    x = np.asarray(x, dtype=np.float32)
    Wq = np.asarray(Wq, dtype=np.float32) * SCALE  # fold softmax scale into Wq
    Wk = np.asarray(Wk, dtype=np.float32)
    Wv = np.asarray(Wv, dtype=np.float32)
    Wo = np.asarray(Wo, dtype=np.float32)

    # RoPE even/odd gather folded into weight column permutation (per head)
    perm128 = np.r_[np.arange(0, 128, 2), np.arange(1, 128, 2)]
    permq = np.concatenate([hb * 128 + perm128 for hb in range(H)])
    permk = np.concatenate([hb * 128 + perm128 for hb in range(KV)])
    Wq_p = Wq[:, permq]
    Wk_p = Wk[:, permk]

    pos = np.arange(T, dtype=np.float64)
    inv_freq = 1.0 / (10000.0 ** (np.arange(0, HD, 2, dtype=np.float64) / HD))
    ang = np.einsum("t,f->tf", pos, inv_freq)  # [T, 64]
    cos = np.cos(ang).T  # [64, T]
    sin = np.sin(ang).T
    cosk = np.concatenate([cos, cos], axis=0).astype(bfloat16)  # [128, T]
    # halves pre-swapped: rows 0:64 hold +sin (odd-half output),
    # rows 64:128 hold -sin (even-half output)
    sink = np.concatenate([sin, -sin], axis=0).astype(bfloat16)

    def sb_layout(w):
        # [D, C] -> [128, NKB*C] matching SBUF tile [128, NKB, C]
        dcols = w.shape[1]
        return np.ascontiguousarray(
            w.reshape(NKB, 128, dcols).transpose(1, 0, 2).reshape(128, NKB * dcols)
        )

    def wq_layout(w):
        # [D, 512] -> [128, NH*NKB*128] head-major matching [128, NH, NKB, 128]
        a = w.reshape(NKB, 128, NH, HD).transpose(1, 2, 0, 3)
        return np.ascontiguousarray(a.reshape(128, NH * NKB * HD))

    def wo_layout(w):
        # [512, D] -> [128, NH*D] matching SBUF tile [128, NH, D]
        return np.ascontiguousarray(
            w.reshape(NH, 128, D).transpose(1, 0, 2).reshape(128, NH * D)
        )

    in_maps = []
    for c in range(8):
        b, s = c // 4, c % 4
        in_maps.append(
            {
                "xT": np.ascontiguousarray(x[b].T).astype(bfloat16),
                "Wq": wq_layout(Wq_p[:, s * 512 : (s + 1) * 512]).astype(bfloat16),
                "Wk": sb_layout(Wk_p[:, s * 128 : (s + 1) * 128]).astype(bfloat16),
                "Wv": sb_layout(Wv[:, s * 128 : (s + 1) * 128]).astype(bfloat16),
                "Wo": wo_layout(Wo[s * 512 : (s + 1) * 512, :]).astype(bfloat16),
                "cosk": cosk,
                "sink": sink,
            }
        )
    return in_maps


def _ensure_ntff_hook():
    """The agent image's antenv lacks axon_hooks, so boot() skips installing
    the NTFF profile hook. Recreate the module and install the hook."""
    import sys
    import types

    try:
        from antenv.axon_hooks import get_axon_ntff_profile_hook  # noqa: F401

        return True
    except ImportError:
        pass
    try:
        import antenv
        from trn_agent_boot.trn_boot import _ntff_profile_via_ctypes

        hook = _ntff_profile_via_ctypes("/opt/axon/libaxon_pjrt.so")
        if hook is None:
            return False
        mod = types.ModuleType("antenv.axon_hooks")
        mod._hook = hook
        mod.set_axon_ntff_profile_hook = lambda h: setattr(mod, "_hook", h)
        mod.get_axon_ntff_profile_hook = lambda: mod._hook
        sys.modules["antenv.axon_hooks"] = mod
        antenv.axon_hooks = mod
        bass_utils.upload_artifacts = lambda d: d
        return True
    except Exception:
        return False


def kernel(x, Wq, Wk, Wv, Wo):
    global _last_results, last_exec_time_ns
    nc = _build_program()
    in_maps = _host_prep(x, Wq, Wk, Wv, Wo)
    trace = bool(int(os.environ.get("KERNEL_TRACE", "0")))
    tmpdir = None
    if trace:
        trace = _ensure_ntff_hook()
        if trace:
            tmpdir = os.environ.get("KERNEL_TRACE_DIR") or None
    res = bass_utils.run_bass_kernel_spmd(
        nc, in_maps, core_ids=list(range(8)), trace=trace, tmpdir=tmpdir
    )
    _last_results = res
    last_exec_time_ns = res.exec_time_ns
    out = np.empty((B, T, D), dtype=np.float32)
    for b in range(B):
        out[b] = sum(
            res.results[4 * b + s]["y"].astype(np.float32) for s in range(TP)
        )
    return out


# revision 37
# speedup vs baseline: 1.0913x; 1.0091x over previous
"""GQA attention (B=2,T=2048,D=2048,H=16,KV=4,HD=128, causal+RoPE) on 8 trn2 cores.

Sharding: 4-way head tensor-parallel x 2-way batch data-parallel.
Core c: batch b=c//4, TP shard s=c%4 -> q heads [4s..4s+3], kv head s.

Transposed-scores flash attention: S^T[k, q] = kT^T qT per 128-key
block x 512-query chunk; exp on the scalar engine writes P^T straight
to SBUF in fp8, so the PV matmul consumes it as the moving operand
(mixed bf16xfp8) with no PE transposes and no PSUM->SBUF copies.
Diagonal blocks compute only their valid q-range (narrowed S^T / exp /
mask / PV / l).  Causal mask = affine_select on GpSimd zeroing the
invalid triangle of P^T in SBUF; diagonal blocks run FIRST in each
unit so the mask chain overlaps the unit's remaining pipeline.
Softmax denominator l: fp8 DoubleRow all-ones matmul per off-diagonal
PT pair (0.5 cycles/row), narrowed fp8 matmuls for diagonal blocks.
1/l on DVE; partition-broadcast on GpSimd; one DVE multiply normalizes
O^T, which feeds the Wo matmul directly as lhsT.  The 1/sqrt(HD) scale
is folded into Wq host-side so q and k share one pair of RoPE tables.
Projection and output-projection work is interleaved unit-by-unit into
the attention stream so the PE fills the slack while the scalar engine
streams exps.  Input DMAs are spread across the sync/gpsimd/scalar
engine queues to parallelize HBM streams.
"""

import math
import os
import numpy as np

try:
    import concourse.bass as bass
except ImportError:  # pragma: no cover
    import sys

    sys.path.insert(0, "/opt/trn_rl_repo")
    import concourse.bass as bass

import concourse.mybir as mybir
import concourse.bacc as bacc
from concourse import bass_utils
from concourse.tile import TileContext
from contextlib import ExitStack
from ml_dtypes import bfloat16

B, T, D = 2, 2048, 2048
H, KV, HD = 16, 4, 128
TP = 4  # head-TP ways
NH = H // TP  # q heads per core = 4
NKB = D // 128  # 16 contraction blocks
NQC = T // 512  # 4 query chunks
NTB = T // 128  # 16 token blocks
SCALE = 1.0 / math.sqrt(HD)
F32 = mybir.dt.float32
BF16 = mybir.dt.bfloat16
FP8 = mybir.dt.float8e4
EXP = mybir.ActivationFunctionType.Exp
DR = mybir.MatmulPerfMode.DoubleRow

_program = None
_last_results = None
last_exec_time_ns = None


def _build_program():
    global _program
    if _program is not None:
        return _program

    nc = bacc.Bacc(
        "TRN2",
        target_bir_lowering=False,
        debug=False,
        enable_asserts=False,
        num_devices=8,
    )
    xT_d = nc.dram_tensor("xT", [D, T], BF16, kind="ExternalInput").ap()
    # weights pre-laid-out host side to match SBUF tiles
    wq_d = nc.dram_tensor("Wq", [128, NH * NKB * HD], BF16, kind="ExternalInput").ap()
    wk_d = nc.dram_tensor("Wk", [128, NKB * HD], BF16, kind="ExternalInput").ap()
    wv_d = nc.dram_tensor("Wv", [128, NKB * HD], BF16, kind="ExternalInput").ap()
    wo_d = nc.dram_tensor("Wo", [128, NH * D], BF16, kind="ExternalInput").ap()
    ck_d = nc.dram_tensor("cosk", [128, T], BF16, kind="ExternalInput").ap()
    sk_d = nc.dram_tensor("sink", [128, T], BF16, kind="ExternalInput").ap()
    y_d = nc.dram_tensor("y", [T, D], BF16, kind="ExternalOutput").ap()

    with TileContext(nc) as tc, ExitStack() as ctx:
        big = ctx.enter_context(tc.tile_pool(name="big", bufs=1))
        ps = ctx.enter_context(tc.tile_pool(name="ps", bufs=2, space="PSUM"))
        stp = ctx.enter_context(tc.tile_pool(name="stp", bufs=3, space="PSUM"))
        otp = ctx.enter_context(tc.tile_pool(name="otp", bufs=2, space="PSUM"))
        lp = ctx.enter_context(tc.tile_pool(name="lp", bufs=1, space="PSUM"))
        ropep = ctx.enter_context(tc.tile_pool(name="ropep", bufs=3))
        ptp = ctx.enter_context(tc.tile_pool(name="ptp", bufs=4))
        rlp = ctx.enter_context(tc.tile_pool(name="rlp", bufs=3))
        yp = ctx.enter_context(tc.tile_pool(name="yp", bufs=4))

        xT = big.tile([128, NKB, T], BF16, tag="xT")
        # one tile per head so each head's first matmul depends only on its
        # own DMA (a shared tile coarsens the dependency to all four DMAs)
        wqh = [
            big.tile([128, NKB, HD], BF16, tag=f"wq{h}", name=f"wq{h}")
            for h in range(NH)
        ]
        wk = big.tile([128, NKB, HD], BF16, tag="wk")
        wv = big.tile([128, NKB, HD], BF16, tag="wv")
        wo = big.tile([128, NH, D], BF16, tag="wo")
        ck = big.tile([128, T], BF16, tag="ck")
        sk = big.tile([128, T], BF16, tag="sk")
        ones8 = big.tile([128, 2, 128], FP8, tag="ones8")
        bias1 = big.tile([128, 1], F32, tag="bias1")
        nc.vector.memset(ones8[:], 1.0)
        nc.vector.memset(bias1[:], -1.0)
        qT = big.tile([128, NH, T], BF16, tag="qT")
        kT = big.tile([128, T], BF16, tag="kT")
        V = big.tile([128, NTB, HD], BF16, tag="V")
        OT = big.tile([128, NH, T], BF16, tag="OT")

        # ---- loads; spread across engine DMA queues so streams parallelize.
        # sync: xT (8 MB).  gpsimd: wk, rope tables, wv, wq (head-major).
        # scalar: wo.
        for q in range(4):
            nc.gpsimd.dma_start(
                out=wk[:, 4 * q : 4 * q + 4, :],
                in_=wk_d[:, q * 512 : (q + 1) * 512],
            )
        # all xT triggers ride the otherwise-idle sync engine: DMA triggers
        # cost ~1us of issuing-engine time, which the scalar engine (exp
        # stream) cannot spare
        for j in range(NKB):
            nc.sync.dma_start(
                out=xT[:, j, 0:512], in_=xT_d[j * 128 : (j + 1) * 128, 0:512]
            )
        for d_ap, s_tile in ((ck_d, ck), (sk_d, sk)):
            nc.gpsimd.dma_start(out=s_tile[:, :1024], in_=d_ap[:, :1024])
            nc.gpsimd.dma_start(out=s_tile[:, 1024:], in_=d_ap[:, 1024:])
        for h in range(NH):
            nc.gpsimd.dma_start(
                out=wqh[h][:, :, :], in_=wq_d[:, h * 2048 : (h + 1) * 2048]
            )
        for q in range(4):
            nc.gpsimd.dma_start(
                out=wv[:, 4 * q : 4 * q + 4, :],
                in_=wv_d[:, q * 512 : (q + 1) * 512],
            )
        for q in range(8):
            nc.scalar.dma_start(
                out=wo[:, q // 2, (q % 2) * 1024 : (q % 2) * 1024 + 1024],
                in_=wo_d[:, q * 1024 : (q + 1) * 1024],
            )
        for j in range(NKB):
            nc.sync.dma_start(
                out=xT[:, j, 512:2048], in_=xT_d[j * 128 : (j + 1) * 128, 512:2048]
            )

        def rope_unit(w_lhsT_of_j, dst, c):
            sl = slice(c * 512, (c + 1) * 512)
            pst = ps.tile([128, 512], F32, tag="ps")
            for j in range(NKB):
                nc.tensor.matmul(
                    pst[:],
                    lhsT=w_lhsT_of_j(j),
                    rhs=xT[:, j, sl],
                    start=(j == 0),
                    stop=(j == NKB - 1),
                )
            ri = ropep.tile([128, 512], BF16, tag="ri")
            nc.scalar.copy(ri[:], pst[:])
            t1 = ropep.tile([128, 512], BF16, tag="t1")
            t2 = ropep.tile([128, 512], BF16, tag="t2")
            # sin table halves pre-swapped host-side so each mul's two SBUF
            # inputs share a base partition (BIR verifier requirement)
            nc.vector.tensor_mul(t1[:], ri[:], ck[:, sl])
            nc.vector.tensor_mul(t2[0:64, :], ri[64:128, :], sk[64:128, sl])
            nc.vector.tensor_mul(t2[64:128, :], ri[0:64, :], sk[0:64, sl])
            nc.vector.tensor_add(dst[:, sl], t1[:], t2[:])

        def v_unit(tb):
            pst = ps.tile([128, 512], F32, tag="ps")
            for j in range(NKB):
                nc.tensor.matmul(
                    pst[:, 0:128],
                    lhsT=xT[:, j, tb * 128 : (tb + 1) * 128],
                    rhs=wv[:, j, :],
                    start=(j == 0),
                    stop=(j == NKB - 1),
                )
            nc.vector.tensor_copy(V[:, tb, :], pst[:, 0:128])

        def proj_units(c):
            units = [lambda: rope_unit(lambda j: wk[:, j, :], kT, c)]
            for h in range(NH):
                units.append(
                    lambda h=h: rope_unit(lambda j: wqh[h][:, j, :], qT[:, h, :], c)
                )
            for tb in range(4 * c, 4 * c + 4):
                units.append(lambda tb=tb: v_unit(tb))
            return units

        def attn_unit(qc, h):
            qsl = slice(qc * 512, (qc + 1) * 512)
            nkb = 4 * (qc + 1)
            # diagonal blocks first so their mask chain overlaps the rest
            order = list(range(4 * qc, nkb)) + list(range(0, 4 * qc))
            ot_ps = otp.tile([128, 512], F32, tag="ot")
            l_ps = lp.tile([128, 512], F32, tag="l")
            pt = None
            for idx, kb in enumerate(order):
                diag = kb >= 4 * qc
                jd = kb - 4 * qc if diag else 0
                off = 128 * jd  # first valid q column within the chunk
                w = 512 - off
                if idx % 2 == 0:
                    pt = ptp.tile([128, 2, 512], FP8, tag="pt")
                psl = pt[:, idx % 2, off:512]
                st = stp.tile([128, 512], F32, tag="st")
                nc.tensor.matmul(
                    st[:, 0:w],
                    lhsT=kT[:, kb * 128 : (kb + 1) * 128],
                    rhs=qT[:, h, qc * 512 + off : (qc + 1) * 512],
                    start=True,
                    stop=True,
                )
                # exp(s - 1): bias keeps P^T in fp8's normal range
                # (cancels in the softmax normalization)
                nc.scalar.activation(psl, st[:, 0:w], EXP, bias=bias1[:])
                if diag:
                    # zero where q < k within the diagonal block
                    nc.gpsimd.affine_select(
                        out=psl,
                        in_=psl,
                        pattern=[[1, w]],
                        compare_op=mybir.AluOpType.is_ge,
                        fill=0.0,
                        base=0,
                        channel_multiplier=-1,
                    )
                nc.tensor.matmul(
                    ot_ps[:, off:512],
                    lhsT=V[:, kb, :],
                    rhs=psl,
                    start=(idx == 0),
                    stop=(idx == nkb - 1),
                )
                if diag:
                    # narrowed fp8 denominator matmul per diagonal block
                    nc.tensor.matmul(
                        l_ps[:, off:512],
                        lhsT=ones8[:, 0, :],
                        rhs=psl,
                        start=(idx == 0),
                        stop=(idx == nkb - 1),
                    )
                elif idx % 2 == 1:
                    # off-diagonal pairs: fp8 DoubleRow all-ones matmul
                    nc.tensor.matmul(
                        l_ps[:],
                        lhsT=ones8[:],
                        rhs=pt[:, :, :],
                        start=False,
                        stop=(idx == nkb - 1),
                        perf_mode=DR,
                    )
            # fast 1-op approx reciprocal (~18 bits, result feeds bf16 anyway);
            # the exact DVE reciprocal is ~3.3us and clogs the vector queue
            rl = rlp.tile([128, 512], F32, tag="rl")
            nc.vector.reciprocal_approx_fast(rl[0:1, :], l_ps[0:1, :])
            bc = rlp.tile([128, 512], F32, tag="bc")
            nc.gpsimd.partition_broadcast(bc[:], rl[0:1, :])
            nc.vector.tensor_mul(OT[:, h, qsl], ot_ps[:], bc[:])

        def wo_unit(tb, dc, last=False):
            yps = ps.tile([128, 512], F32, tag="ps")
            for h in range(NH):
                nc.tensor.matmul(
                    yps[:],
                    lhsT=OT[:, h, tb * 128 : (tb + 1) * 128],
                    rhs=wo[:, h, dc * 512 : (dc + 1) * 512],
                    start=(h == 0),
                    stop=(h == NH - 1),
                )
            ysb = yp.tile([128, 512], BF16, tag="y")
            if dc % 2 == 0:
                nc.vector.tensor_copy(ysb[:], yps[:])
            else:
                nc.scalar.copy(ysb[:], yps[:])
            # stores stay off the gpsimd queue (it carries the mask /
            # broadcast chain); the final chunk splits each store across
            # two queues to shorten the drain tail
            if last:
                for sp, eng in ((0, nc.sync), (1, nc.scalar)):
                    eng.dma_start(
                        out=y_d[
                            tb * 128 : (tb + 1) * 128,
                            dc * 512 + sp * 256 : dc * 512 + sp * 256 + 256,
                        ],
                        in_=ysb[:, sp * 256 : sp * 256 + 256],
                    )
            else:
                nc.sync.dma_start(
                    out=y_d[tb * 128 : (tb + 1) * 128, dc * 512 : (dc + 1) * 512],
                    in_=ysb[:],
                )

        # interleave: after each attention unit, emit a couple of next-chunk
        # projection units and one token-block of previous-chunk Wo units
        PROJ_SPLIT = [2, 2, 2, 3]  # 9 proj units spread over 4 attn units
        for u in proj_units(0):
            u()
        for qc in range(NQC):
            fillers = proj_units(qc + 1) if qc + 1 < NQC else []
            fi = 0
            for h in range(NH):
                attn_unit(qc, h)
                for _ in range(PROJ_SPLIT[h]):
                    if fi < len(fillers):
                        fillers[fi]()
                        fi += 1
                if qc > 0:
                    tb = 4 * (qc - 1) + h
                    for dc in range(4):
                        wo_unit(tb, dc)
            while fi < len(fillers):
                fillers[fi]()
                fi += 1
        for tb in range(12, 16):
            for dc in range(4):
                wo_unit(tb, dc, last=True)

    nc.compile()
    _program = nc
    return nc


def _host_prep(x, Wq, Wk, Wv, Wo):
    x = np.asarray(x, dtype=np.float32)
    Wq = np.asarray(Wq, dtype=np.float32) * SCALE  # fold softmax scale into Wq
    Wk = np.asarray(Wk, dtype=np.float32)
    Wv = np.asarray(Wv, dtype=np.float32)
    Wo = np.asarray(Wo, dtype=np.float32)

    # RoPE even/odd gather folded into weight column permutation (per head)
    perm128 = np.r_[np.arange(0, 128, 2), np.arange(1, 128, 2)]
    permq = np.concatenate([hb * 128 + perm128 for hb in range(H)])
    permk = np.concatenate([hb * 128 + perm128 for hb in range(KV)])
    Wq_p = Wq[:, permq]
    Wk_p = Wk[:, permk]

    pos = np.arange(T, dtype=np.float64)
    inv_freq = 1.0 / (10000.0 ** (np.arange(0, HD, 2, dtype=np.float64) / HD))
    ang = np.einsum("t,f->tf", pos, inv_freq)  # [T, 64]
    cos = np.cos(ang).T  # [64, T]
    sin = np.sin(ang).T
    cosk = np.concatenate([cos, cos], axis=0).astype(bfloat16)  # [128, T]
    # halves pre-swapped: rows 0:64 hold +sin (odd-half output),
    # rows 64:128 hold -sin (even-half output)
    sink = np.concatenate([sin, -sin], axis=0).astype(bfloat16)

    def sb_layout(w):
        # [D, C] -> [128, NKB*C] matching SBUF tile [128, NKB, C]
        dcols = w.shape[1]
        return np.ascontiguousarray(
            w.reshape(NKB, 128, dcols).transpose(1, 0, 2).reshape(128, NKB * dcols)
        )

    def wq_layout(w):
        # [D, 512] -> [128, NH*NKB*128] head-major matching [128, NH, NKB, 128]
        a = w.reshape(NKB, 128, NH, HD).transpose(1, 2, 0, 3)
        return np.ascontiguousarray(a.reshape(128, NH * NKB * HD))

    def wo_layout(w):
        # [512, D] -> [128, NH*D] matching SBUF tile [128, NH, D]
        return np.ascontiguousarray(
            w.reshape(NH, 128, D).transpose(1, 0, 2).reshape(128, NH * D)
        )

    in_maps = []
    for c in range(8):
        b, s = c // 4, c % 4
        in_maps.append(
            {
                "xT": np.ascontiguousarray(x[b].T).astype(bfloat16),
                "Wq": wq_layout(Wq_p[:, s * 512 : (s + 1) * 512]).astype(bfloat16),
                "Wk": sb_layout(Wk_p[:, s * 128 : (s + 1) * 128]).astype(bfloat16),
                "Wv": sb_layout(Wv[:, s * 128 : (s + 1) * 128]).astype(bfloat16),
                "Wo": wo_layout(Wo[s * 512 : (s + 1) * 512, :]).astype(bfloat16),
                "cosk": cosk,
                "sink": sink,
            }
        )
    return in_maps


def _ensure_ntff_hook():
    """The agent image's antenv lacks axon_hooks, so boot() skips installing
    the NTFF profile hook. Recreate the module and install the hook."""
    import sys
    import types

    try:
        from antenv.axon_hooks import get_axon_ntff_profile_hook  # noqa: F401

        return True
    except ImportError:
        pass
    try:
        import antenv
        from trn_agent_boot.trn_boot import _ntff_profile_via_ctypes

        hook = _ntff_profile_via_ctypes("/opt/axon/libaxon_pjrt.so")
        if hook is None:
            return False
        mod = types.ModuleType("antenv.axon_hooks")
        mod._hook = hook
        mod.set_axon_ntff_profile_hook = lambda h: setattr(mod, "_hook", h)
        mod.get_axon_ntff_profile_hook = lambda: mod._hook
        sys.modules["antenv.axon_hooks"] = mod
        antenv.axon_hooks = mod
        bass_utils.upload_artifacts = lambda d: d
        return True
    except Exception:
        return False


def kernel(x, Wq, Wk, Wv, Wo):
    global _last_results, last_exec_time_ns
    nc = _build_program()
    in_maps = _host_prep(x, Wq, Wk, Wv, Wo)
    trace = bool(int(os.environ.get("KERNEL_TRACE", "0")))
    tmpdir = None
    if trace:
        trace = _ensure_ntff_hook()
        if trace:
            tmpdir = os.environ.get("KERNEL_TRACE_DIR") or None
    res = bass_utils.run_bass_kernel_spmd(
        nc, in_maps, core_ids=list(range(8)), trace=trace, tmpdir=tmpdir
    )
    _last_results = res
    last_exec_time_ns = res.exec_time_ns
    out = np.empty((B, T, D), dtype=np.float32)
    for b in range(B):
        out[b] = sum(
            res.results[4 * b + s]["y"].astype(np.float32) for s in range(TP)
        )
    return out
